# revision 41
# baseline (speedup 1.0000x reference)
"""MoE linear (modality-routed) Trainium2 kernel.

out[n] = x[n] @ W[modality_ids[n]].T + b[modality_ids[n]]

Strategy (data parallel over 8 cores, weight replicated):
- Host: per core shard of 16384 tokens, stable-argsort tokens by expert.
  Groups padded to a shared per-expert capacity (multiple of 128) so one
  SPMD NEFF serves all cores; per-tile expert is a compile-time constant.
- Device: x is host-cast to bf16 (halves gather traffic; W/PSUM/bias
  stay f32). Input side uses batched dma_gather (one Pool instruction
  per G=8 128-token tiles, int16 indices wrap-16 across partitions);
  per tile: PE transpose -> copy to SBUF on the Activation engine -> 4
  accumulating fp32r matmuls against SBUF-resident W^T -> bias add on
  DVE (cast to bf16) into a per-batch output tile. One regular HWDGE
  store per batch writes the batch to a dense expert-sorted scratch
  ys (disjoint regions, fully parallel). The host applies the inverse
  permutation (and bf16->f32 upcast) while unsharding — the sorted
  batch store interleaves tokens as row = t0*128 + p*g + j, which the
  host index map accounts for.
"""

import sys

if "/opt/trn_rl_repo" not in sys.path:
    sys.path.insert(0, "/opt/trn_rl_repo")

import numpy as np
from ml_dtypes import bfloat16

import concourse.bass as bass  # noqa: F401
import concourse.tile as tile
from concourse import bacc, mybir
from concourse.bass import IndirectOffsetOnAxis
from concourse.bass_utils import run_bass_kernel_spmd

N_CORES = 8
N_TOKENS = 131072
N_SHARD = N_TOKENS // N_CORES  # 16384
D_IN = 512
D_OUT = 512
N_EXPERTS = 3
P = 128
KC = D_IN // P  # 4 contraction chunks
G = 8  # tiles per dma_gather batch

_NC_CACHE = {}


def build_nc(n_shard, caps, num_devices=N_CORES):
    """Build + compile the SPMD Bass kernel for given per-expert capacities."""
    key = (n_shard, tuple(caps), num_devices)
    if key in _NC_CACHE:
        return _NC_CACHE[key]
    npad = sum(caps)
    nt = npad // P
    experts_of_tile = []
    for e, c in enumerate(caps):
        experts_of_tile += [e] * (c // P)

    nc = bacc.Bacc(
        "TRN2", target_bir_lowering=False, debug=False, num_devices=num_devices
    )
    f32 = mybir.dt.float32
    f32r = mybir.dt.float32r
    bf16 = mybir.dt.bfloat16
    i16 = mybir.dt.int16
    i32 = mybir.dt.int32

    # x is fed as bf16 (host-cast): halves gather traffic and speeds the PE
    # transposes; W/PSUM/bias/y stay f32.
    x = nc.dram_tensor("x", [n_shard, D_IN], bf16, kind="ExternalInput").ap()
    wt = nc.dram_tensor(
        "wt", [D_IN, N_EXPERTS * D_OUT], f32r, kind="ExternalInput"
    ).ap()
    bb = nc.dram_tensor(
        "bias_bc", [P, N_EXPERTS * D_OUT], f32, kind="ExternalInput"
    ).ap()
    gidx = nc.dram_tensor("gidx", [P, npad // 16], i16, kind="ExternalInput").ap()
    idn = nc.dram_tensor("idn", [P, P], bf16, kind="ExternalInput").ap()
    ys = nc.dram_tensor("ys", [npad, D_OUT], bf16, kind="ExternalOutput").ap()

    with tile.TileContext(nc) as tc:
        with (
            tc.tile_pool(name="const", bufs=1) as cpool,
            tc.tile_pool(name="xg", bufs=7) as xg_pool,
            tc.tile_pool(name="xt", bufs=5) as xt_pool,
            tc.tile_pool(name="outp", bufs=8) as out_pool,
            tc.tile_pool(name="ptr", bufs=3, space="PSUM") as ptr_pool,
            tc.tile_pool(name="warmp", bufs=1, space="PSUM") as warm_pool,
            tc.tile_pool(name="pmm", bufs=4, space="PSUM") as pmm_pool,
        ):
            # Routing tables first: they gate the first gather, while
            # weights are only needed by the first matmul ~10us later. The
            # identity comes from the host (make_identity would occupy the
            # Pool engine ahead of the first gather's descriptor gen).
            gidx_sb = cpool.tile([P, npad // 16], i16)
            # First half-batch's index columns first (8 KB) so the first
            # gather's descriptor gen isn't gated on the full 260 KB table.
            nc.sync.dma_start(out=gidx_sb[:, 0:32], in_=gidx[:, 0:32])
            nc.sync.dma_start(out=gidx_sb[:, 32:], in_=gidx[:, 32:])
            ident = cpool.tile([P, P], bf16)
            nc.sync.dma_start(out=ident[:], in_=idn[:])
            bias_sb = cpool.tile([P, N_EXPERTS * D_OUT], f32)
            nc.sync.dma_start(out=bias_sb[:], in_=bb[:])
            # W^T resident in SBUF: block (e, kc) is [k=128, o=512]
            w_sb = cpool.tile([P, N_EXPERTS * KC * D_OUT], f32r)
            for e in range(N_EXPERTS):
                for kc in range(KC):
                    nc.sync.dma_start(
                        out=w_sb[:, (e * KC + kc) * D_OUT : (e * KC + kc + 1) * D_OUT],
                        in_=wt[kc * P : (kc + 1) * P, e * D_OUT : (e + 1) * D_OUT],
                    )
            # Warm the PE pstate ramp during the first gather's round
            # trip: dummy transposes of the identity into a scratch PSUM
            # tile (never read), back-to-back from ~t=3us. Un-ramped PE
            # runs transposes ~4x and f32r matmuls ~8x slower.
            warm = warm_pool.tile([P, P], bf16)
            for _ in range(24):
                nc.tensor.transpose(warm[:], ident[:], ident[:])

            # Matmuls are emitted MM_DELAY tiles behind transposes, so the
            # PE sequencer (head-of-line) never waits on the Act-engine
            # PSUM->SBUF copy; the wait is hidden under later transposes.
            MM_DELAY = 3
            mm_pending = []
            st_pending = []

            last_t0 = ((nt - 1) // G) * G

            def emit_matmul(t, xt, osb, j):
                e = experts_of_tile[t]
                pmm = pmm_pool.tile([P, D_OUT], f32)
                for kc in range(KC):
                    nc.tensor.matmul(
                        pmm[:],
                        lhsT=xt[:, kc * P : (kc + 1) * P],
                        rhs=w_sb[:, (e * KC + kc) * D_OUT : (e * KC + kc + 1) * D_OUT],
                        start=(kc == 0),
                        stop=(kc == KC - 1),
                    )
                nc.vector.tensor_add(
                    out=osb[:, j, :],
                    in0=pmm[:],
                    in1=bias_sb[:, e * D_OUT : (e + 1) * D_OUT],
                )
                if t >= last_t0:
                    # Last batch: per-tile stores (natural row order) so the
                    # final transfer doesn't serialize the drain.
                    nc.sync.dma_start(
                        out=ys[t * P : (t + 1) * P, :], in_=osb[:, j, :]
                    )

            for t0 in range(0, nt, G):
                g = min(G, nt - t0)
                # Batched gather: xg[p, j, :] = x[idxs[j*128+p]] where idxs
                # covers sorted slots [t0*128, (t0+g)*128).
                xg = xg_pool.tile([P, g, D_IN], bf16)
                if t0 == 0 and g == G:
                    # Split the first gather so the PE's first transpose
                    # starts half a transfer earlier.
                    h = G // 2
                    for s in range(2):
                        nc.gpsimd.dma_gather(
                            xg[:, s * h : (s + 1) * h, :],
                            x[:],
                            gidx_sb[:, s * h * (P // 16) : (s + 1) * h * (P // 16)],
                            h * P,
                            h * P,
                            D_IN,
                        )
                else:
                    nc.gpsimd.dma_gather(
                        xg[:],
                        x[:],
                        gidx_sb[:, t0 * (P // 16) : (t0 + g) * (P // 16)],
                        g * P,
                        g * P,
                        D_IN,
                    )
                osb = out_pool.tile([P, g, D_OUT], bf16)
                for j in range(g):
                    t = t0 + j
                    ptr = ptr_pool.tile([P, D_IN], bf16)
                    for kc in range(KC):
                        nc.tensor.transpose(
                            ptr[:, kc * P : (kc + 1) * P],
                            xg[:, j, kc * P : (kc + 1) * P],
                            ident[:],
                        )
                    xt = xt_pool.tile([P, D_IN], f32r)
                    nc.scalar.copy(xt[:], ptr[:])
                    mm_pending.append((t, xt, osb, j))
                    if len(mm_pending) > MM_DELAY:
                        emit_matmul(*mm_pending.pop(0))
                # Dense batch store: DRAM row t0*128 + p*g + j <- osb[p, j, :]
                # (DMA pairs the row walk with partition-major SBUF order).
                # Deferred one batch so the MM_DELAY-deferred adds of this
                # batch's last tiles are emitted before the store (else the
                # dep tracker can't chain them and the store races).
                if t0 < last_t0:
                    st_pending.append((t0, g, osb))
                if len(st_pending) > 1:
                    st0, sg, sosb = st_pending.pop(0)
                    nc.sync.dma_start(
                        out=ys[st0 * P : (st0 + sg) * P, :], in_=sosb[:]
                    )
            # Flush: the second-to-last batch's adds already exist, so its
            # store can go before the tail batch's deferred matmuls rather
            # than serializing at the very end.
            if len(st_pending) > 1:
                st0, sg, sosb = st_pending.pop(0)
                nc.sync.dma_start(out=ys[st0 * P : (st0 + sg) * P, :], in_=sosb[:])
            for t, xt, osb, j in mm_pending:
                emit_matmul(t, xt, osb, j)
            for st0, sg, sosb in st_pending:
                nc.sync.dma_start(out=ys[st0 * P : (st0 + sg) * P, :], in_=sosb[:])

    nc.compile()
    _NC_CACHE[key] = nc
    return nc


def make_routing(ids_shard, caps):
    """Per-core routing: gidx [P, npad//16] int16 dma_gather indices
    (wrap-16 per G-tile batch, replicated on 8x16 partitions; padding
    gathers row 0) and gd [npad] int64 slot -> original row (-1 padding).
    """
    n_shard = ids_shard.shape[0]
    npad = sum(caps)
    nt = npad // P
    order = np.argsort(ids_shard, kind="stable").astype(np.int64)
    cnt = np.bincount(ids_shard, minlength=N_EXPERTS)
    gs = np.zeros(npad, np.int64)
    gd = np.full(npad, -1, np.int64)
    base = 0
    off = 0
    for e in range(N_EXPERTS):
        c = int(cnt[e])
        seg = order[off : off + c]
        gs[base : base + c] = seg
        gd[base : base + c] = seg
        base += caps[e]
        off += c
    blocks = []
    for t0 in range(0, nt, G):
        g = min(G, nt - t0)
        blk = gs[t0 * P : (t0 + g) * P]
        blocks.append(np.ascontiguousarray(blk.reshape(-1, 16).T))
    gidx = np.tile(np.concatenate(blocks, axis=1), (8, 1)).astype(np.int16)
    return gidx, gd


def ys_row_of_slot(caps):
    """ys row index for each sorted slot s = t*128 + p: the per-batch store
    interleaves as row = t0*128 + p*g + j (j = t - t0)."""
    npad = sum(caps)
    nt = npad // P
    rows = np.empty(npad, np.int64)
    last_t0 = ((nt - 1) // G) * G
    for t0 in range(0, nt, G):
        g = min(G, nt - t0)
        j = np.arange(g)[None, :]
        p = np.arange(P)[:, None]
        slots = ((t0 + j) * P + p).ravel()
        if t0 == last_t0:
            # last batch stored per-tile: natural row order
            rows[slots] = slots
        else:
            # slot (t0+j)*128 + p  ->  row t0*128 + p*g + j
            rows[slots] = (t0 * P + p * g + j).ravel()
    return rows


def prepare(inputs):
    """Shared host-side prep: returns (nc, in_maps)."""
    x = np.ascontiguousarray(np.asarray(inputs["x"], dtype=np.float32))
    ids = np.asarray(inputs["modality_ids"]).astype(np.int64)
    weight = np.asarray(inputs["weight"], dtype=np.float32)
    b = np.asarray(inputs["bias"], dtype=np.float32)

    wt = np.ascontiguousarray(weight.T)  # [D_IN, E*D_OUT]
    bias_bc = np.ascontiguousarray(
        np.broadcast_to(b[None, :], (P, N_EXPERTS * D_OUT))
    )

    counts = np.stack(
        [
            np.bincount(ids[c * N_SHARD : (c + 1) * N_SHARD], minlength=N_EXPERTS)
            for c in range(N_CORES)
        ]
    )
    caps = [int(-(-counts[:, e].max() // P) * P) for e in range(N_EXPERTS)]

    nc = build_nc(N_SHARD, caps)
    in_maps = []
    gds = []
    for c in range(N_CORES):
        ids_c = ids[c * N_SHARD : (c + 1) * N_SHARD]
        gidx, gd = make_routing(ids_c, caps)
        gds.append(gd)
        in_maps.append(
            {
                "x": np.ascontiguousarray(
                    x[c * N_SHARD : (c + 1) * N_SHARD].astype(bfloat16)
                ),
                "wt": wt,
                "bias_bc": bias_bc,
                "gidx": gidx,
                "idn": np.eye(P, dtype=bfloat16),
            }
        )
    return nc, in_maps, gds, caps


def run(inputs, trace=False):
    """Returns (out, BassKernelResults)."""
    nc, in_maps, gds, caps = prepare(inputs)
    res = run_bass_kernel_spmd(nc, in_maps, list(range(N_CORES)), trace=trace)
    rows = ys_row_of_slot(caps)
    out = np.empty((N_TOKENS, D_OUT), np.float32)
    for c in range(N_CORES):
        gd = gds[c]
        valid = gd >= 0
        ys = np.asarray(res.results[c]["ys"])
        # inverse permutation + bf16 -> f32 upcast during unshard
        out[c * N_SHARD + gd[valid]] = ys[rows[valid]].astype(np.float32)
    return out, res


def kernel(**inputs):
    out, _ = run(inputs, trace=False)
    return out


# revision 42
# speedup vs baseline: 1.0033x; 1.0033x over previous
"""MoE linear (modality-routed) Trainium2 kernel.

out[n] = x[n] @ W[modality_ids[n]].T + b[modality_ids[n]]

Strategy (data parallel over 8 cores, weight replicated):
- Host: per core shard of 16384 tokens, stable-argsort tokens by expert.
  Groups padded to a shared per-expert capacity (multiple of 128) so one
  SPMD NEFF serves all cores; per-tile expert is a compile-time constant.
- Device: x is host-cast to bf16 (halves gather traffic; W/PSUM/bias
  stay f32). Input side uses batched dma_gather (one Pool instruction
  per G=8 128-token tiles, int16 indices wrap-16 across partitions);
  per tile: PE transpose -> copy to SBUF on the Activation engine -> 4
  accumulating fp32r matmuls against SBUF-resident W^T -> bias add on
  DVE (cast to bf16) into a per-batch output tile. One regular HWDGE
  store per batch writes the batch to a dense expert-sorted scratch
  ys (disjoint regions, fully parallel). The host applies the inverse
  permutation (and bf16->f32 upcast) while unsharding — the sorted
  batch store interleaves tokens as row = t0*128 + p*g + j, which the
  host index map accounts for.
"""

import sys

if "/opt/trn_rl_repo" not in sys.path:
    sys.path.insert(0, "/opt/trn_rl_repo")

import numpy as np
from ml_dtypes import bfloat16

import concourse.bass as bass  # noqa: F401
import concourse.tile as tile
from concourse import bacc, mybir
from concourse.bass import IndirectOffsetOnAxis
from concourse.bass_utils import run_bass_kernel_spmd

N_CORES = 8
N_TOKENS = 131072
N_SHARD = N_TOKENS // N_CORES  # 16384
D_IN = 512
D_OUT = 512
N_EXPERTS = 3
P = 128
KC = D_IN // P  # 4 contraction chunks
G = 8  # tiles per dma_gather batch

_NC_CACHE = {}


def build_nc(n_shard, caps, num_devices=N_CORES):
    """Build + compile the SPMD Bass kernel for given per-expert capacities."""
    key = (n_shard, tuple(caps), num_devices)
    if key in _NC_CACHE:
        return _NC_CACHE[key]
    npad = sum(caps)
    nt = npad // P
    experts_of_tile = []
    for e, c in enumerate(caps):
        experts_of_tile += [e] * (c // P)

    nc = bacc.Bacc(
        "TRN2", target_bir_lowering=False, debug=False, num_devices=num_devices
    )
    f32 = mybir.dt.float32
    f32r = mybir.dt.float32r
    bf16 = mybir.dt.bfloat16
    i16 = mybir.dt.int16
    i32 = mybir.dt.int32

    # x is fed as bf16 (host-cast): halves gather traffic and speeds the PE
    # transposes; W/PSUM/bias/y stay f32.
    x = nc.dram_tensor("x", [n_shard, D_IN], bf16, kind="ExternalInput").ap()
    wt = nc.dram_tensor(
        "wt", [D_IN, N_EXPERTS * D_OUT], f32r, kind="ExternalInput"
    ).ap()
    bb = nc.dram_tensor(
        "bias_bc", [P, N_EXPERTS * D_OUT], f32, kind="ExternalInput"
    ).ap()
    gidx = nc.dram_tensor("gidx", [P, npad // 16], i16, kind="ExternalInput").ap()
    idn = nc.dram_tensor("idn", [P, P], bf16, kind="ExternalInput").ap()
    ys = nc.dram_tensor("ys", [npad, D_OUT], bf16, kind="ExternalOutput").ap()

    with tile.TileContext(nc) as tc:
        with (
            tc.tile_pool(name="const", bufs=1) as cpool,
            tc.tile_pool(name="xg", bufs=7) as xg_pool,
            tc.tile_pool(name="xt", bufs=5) as xt_pool,
            tc.tile_pool(name="outp", bufs=8) as out_pool,
            tc.tile_pool(name="ptr", bufs=3, space="PSUM") as ptr_pool,
            tc.tile_pool(name="warmp", bufs=1, space="PSUM") as warm_pool,
            tc.tile_pool(name="pmm", bufs=4, space="PSUM") as pmm_pool,
        ):
            # Routing tables first: they gate the first gather, while
            # weights are only needed by the first matmul ~10us later. The
            # identity comes from the host (make_identity would occupy the
            # Pool engine ahead of the first gather's descriptor gen).
            gidx_sb = cpool.tile([P, npad // 16], i16)
            # First half-batch's index columns first (8 KB) so the first
            # gather's descriptor gen isn't gated on the full 260 KB table.
            nc.sync.dma_start(out=gidx_sb[:, 0:32], in_=gidx[:, 0:32])
            nc.sync.dma_start(out=gidx_sb[:, 32:], in_=gidx[:, 32:])
            ident = cpool.tile([P, P], bf16)
            nc.sync.dma_start(out=ident[:], in_=idn[:])
            bias_sb = cpool.tile([P, N_EXPERTS * D_OUT], f32)
            nc.sync.dma_start(out=bias_sb[:], in_=bb[:])
            # W^T resident in SBUF: block (e, kc) is [k=128, o=512]
            w_sb = cpool.tile([P, N_EXPERTS * KC * D_OUT], f32r)
            for e in range(N_EXPERTS):
                for kc in range(KC):
                    nc.sync.dma_start(
                        out=w_sb[:, (e * KC + kc) * D_OUT : (e * KC + kc + 1) * D_OUT],
                        in_=wt[kc * P : (kc + 1) * P, e * D_OUT : (e + 1) * D_OUT],
                    )
            # Warm the PE pstate ramp during the first gather's round
            # trip: dummy transposes of the identity into a scratch PSUM
            # tile (never read), back-to-back from ~t=3us. Un-ramped PE
            # runs transposes ~4x and f32r matmuls ~8x slower.
            warm = warm_pool.tile([P, P], bf16)
            for _ in range(10):
                nc.tensor.transpose(warm[:], ident[:], ident[:])

            # Matmuls are emitted MM_DELAY tiles behind transposes, so the
            # PE sequencer (head-of-line) never waits on the Act-engine
            # PSUM->SBUF copy; the wait is hidden under later transposes.
            MM_DELAY = 3
            mm_pending = []
            st_pending = []

            last_t0 = ((nt - 1) // G) * G

            def emit_matmul(t, xt, osb, j):
                e = experts_of_tile[t]
                pmm = pmm_pool.tile([P, D_OUT], f32)
                for kc in range(KC):
                    nc.tensor.matmul(
                        pmm[:],
                        lhsT=xt[:, kc * P : (kc + 1) * P],
                        rhs=w_sb[:, (e * KC + kc) * D_OUT : (e * KC + kc + 1) * D_OUT],
                        start=(kc == 0),
                        stop=(kc == KC - 1),
                    )
                nc.vector.tensor_add(
                    out=osb[:, j, :],
                    in0=pmm[:],
                    in1=bias_sb[:, e * D_OUT : (e + 1) * D_OUT],
                )
                if t >= last_t0:
                    # Last batch: per-tile stores (natural row order) so the
                    # final transfer doesn't serialize the drain.
                    nc.sync.dma_start(
                        out=ys[t * P : (t + 1) * P, :], in_=osb[:, j, :]
                    )

            for t0 in range(0, nt, G):
                g = min(G, nt - t0)
                # Batched gather: xg[p, j, :] = x[idxs[j*128+p]] where idxs
                # covers sorted slots [t0*128, (t0+g)*128).
                xg = xg_pool.tile([P, g, D_IN], bf16)
                if t0 == 0 and g == G:
                    # Split the first gather so the PE's first transpose
                    # starts half a transfer earlier.
                    h = G // 2
                    for s in range(2):
                        nc.gpsimd.dma_gather(
                            xg[:, s * h : (s + 1) * h, :],
                            x[:],
                            gidx_sb[:, s * h * (P // 16) : (s + 1) * h * (P // 16)],
                            h * P,
                            h * P,
                            D_IN,
                        )
                else:
                    nc.gpsimd.dma_gather(
                        xg[:],
                        x[:],
                        gidx_sb[:, t0 * (P // 16) : (t0 + g) * (P // 16)],
                        g * P,
                        g * P,
                        D_IN,
                    )
                osb = out_pool.tile([P, g, D_OUT], bf16)
                for j in range(g):
                    t = t0 + j
                    ptr = ptr_pool.tile([P, D_IN], bf16)
                    for kc in range(KC):
                        nc.tensor.transpose(
                            ptr[:, kc * P : (kc + 1) * P],
                            xg[:, j, kc * P : (kc + 1) * P],
                            ident[:],
                        )
                    xt = xt_pool.tile([P, D_IN], f32r)
                    nc.scalar.copy(xt[:], ptr[:])
                    mm_pending.append((t, xt, osb, j))
                    if len(mm_pending) > MM_DELAY:
                        emit_matmul(*mm_pending.pop(0))
                # Dense batch store: DRAM row t0*128 + p*g + j <- osb[p, j, :]
                # (DMA pairs the row walk with partition-major SBUF order).
                # Deferred one batch so the MM_DELAY-deferred adds of this
                # batch's last tiles are emitted before the store (else the
                # dep tracker can't chain them and the store races).
                if t0 < last_t0:
                    st_pending.append((t0, g, osb))
                if len(st_pending) > 1:
                    st0, sg, sosb = st_pending.pop(0)
                    nc.sync.dma_start(
                        out=ys[st0 * P : (st0 + sg) * P, :], in_=sosb[:]
                    )
            # Flush: the second-to-last batch's adds already exist, so its
            # store can go before the tail batch's deferred matmuls rather
            # than serializing at the very end.
            if len(st_pending) > 1:
                st0, sg, sosb = st_pending.pop(0)
                nc.sync.dma_start(out=ys[st0 * P : (st0 + sg) * P, :], in_=sosb[:])
            for t, xt, osb, j in mm_pending:
                emit_matmul(t, xt, osb, j)
            for st0, sg, sosb in st_pending:
                nc.sync.dma_start(out=ys[st0 * P : (st0 + sg) * P, :], in_=sosb[:])

    nc.compile()
    _NC_CACHE[key] = nc
    return nc


def make_routing(ids_shard, caps):
    """Per-core routing: gidx [P, npad//16] int16 dma_gather indices
    (wrap-16 per G-tile batch, replicated on 8x16 partitions; padding
    gathers row 0) and gd [npad] int64 slot -> original row (-1 padding).
    """
    n_shard = ids_shard.shape[0]
    npad = sum(caps)
    nt = npad // P
    order = np.argsort(ids_shard, kind="stable").astype(np.int64)
    cnt = np.bincount(ids_shard, minlength=N_EXPERTS)
    gs = np.zeros(npad, np.int64)
    gd = np.full(npad, -1, np.int64)
    base = 0
    off = 0
    for e in range(N_EXPERTS):
        c = int(cnt[e])
        seg = order[off : off + c]
        gs[base : base + c] = seg
        gd[base : base + c] = seg
        base += caps[e]
        off += c
    blocks = []
    for t0 in range(0, nt, G):
        g = min(G, nt - t0)
        blk = gs[t0 * P : (t0 + g) * P]
        blocks.append(np.ascontiguousarray(blk.reshape(-1, 16).T))
    gidx = np.tile(np.concatenate(blocks, axis=1), (8, 1)).astype(np.int16)
    return gidx, gd


def ys_row_of_slot(caps):
    """ys row index for each sorted slot s = t*128 + p: the per-batch store
    interleaves as row = t0*128 + p*g + j (j = t - t0)."""
    npad = sum(caps)
    nt = npad // P
    rows = np.empty(npad, np.int64)
    last_t0 = ((nt - 1) // G) * G
    for t0 in range(0, nt, G):
        g = min(G, nt - t0)
        j = np.arange(g)[None, :]
        p = np.arange(P)[:, None]
        slots = ((t0 + j) * P + p).ravel()
        if t0 == last_t0:
            # last batch stored per-tile: natural row order
            rows[slots] = slots
        else:
            # slot (t0+j)*128 + p  ->  row t0*128 + p*g + j
            rows[slots] = (t0 * P + p * g + j).ravel()
    return rows


def prepare(inputs):
    """Shared host-side prep: returns (nc, in_maps)."""
    x = np.ascontiguousarray(np.asarray(inputs["x"], dtype=np.float32))
    ids = np.asarray(inputs["modality_ids"]).astype(np.int64)
    weight = np.asarray(inputs["weight"], dtype=np.float32)
    b = np.asarray(inputs["bias"], dtype=np.float32)

    wt = np.ascontiguousarray(weight.T)  # [D_IN, E*D_OUT]
    bias_bc = np.ascontiguousarray(
        np.broadcast_to(b[None, :], (P, N_EXPERTS * D_OUT))
    )

    counts = np.stack(
        [
            np.bincount(ids[c * N_SHARD : (c + 1) * N_SHARD], minlength=N_EXPERTS)
            for c in range(N_CORES)
        ]
    )
    caps = [int(-(-counts[:, e].max() // P) * P) for e in range(N_EXPERTS)]

    nc = build_nc(N_SHARD, caps)
    in_maps = []
    gds = []
    for c in range(N_CORES):
        ids_c = ids[c * N_SHARD : (c + 1) * N_SHARD]
        gidx, gd = make_routing(ids_c, caps)
        gds.append(gd)
        in_maps.append(
            {
                "x": np.ascontiguousarray(
                    x[c * N_SHARD : (c + 1) * N_SHARD].astype(bfloat16)
                ),
                "wt": wt,
                "bias_bc": bias_bc,
                "gidx": gidx,
                "idn": np.eye(P, dtype=bfloat16),
            }
        )
    return nc, in_maps, gds, caps


def run(inputs, trace=False):
    """Returns (out, BassKernelResults)."""
    nc, in_maps, gds, caps = prepare(inputs)
    res = run_bass_kernel_spmd(nc, in_maps, list(range(N_CORES)), trace=trace)
    rows = ys_row_of_slot(caps)
    out = np.empty((N_TOKENS, D_OUT), np.float32)
    for c in range(N_CORES):
        gd = gds[c]
        valid = gd >= 0
        ys = np.asarray(res.results[c]["ys"])
        # inverse permutation + bf16 -> f32 upcast during unshard
        out[c * N_SHARD + gd[valid]] = ys[rows[valid]].astype(np.float32)
    return out, res


def kernel(**inputs):
    out, _ = run(inputs, trace=False)
    return out
